# revision 9
# baseline (speedup 1.0000x reference)
"""Multi-head causal self-attention on 8 Trainium2 NeuronCores.

Sharding: core = (batch b, head-half). Each of the 8 cores computes
attention for 8 of the 16 heads of one of the 4 batch elements, plus the
partial output projection over its 512 feature columns. Host sums the two
partial projections per batch and adds the bias.

All device tensors are kept transposed (feature-major) so every matmul
contraction lands on the partition axis:
  QK^T:  S^T[k,q] = K^T_blk.T @ Q^T_chunk           (contraction 64)
  AV:    outT[d,q] = V_ext_blk.T @ expS^T_blk       (contraction k=128)

QK^T runs both heads of a head-pair as two CONCURRENT row-tiled K=64
matmuls (head A in PE rows 0:64, head B in rows 64:128, tile_position
auto-derived from the operands' base partitions) writing the two halves
of one [128, 2, 512] PSUM stag tile — 2x the padded-K=128 scheme.
Diagonal-band tiles only compute/exp the live columns [off:512].

Engine-queue discipline (each queue is in-order; an op that waits on a
semaphore blocks everything behind it):
  ACT    exp only (the attention critical path).
  DVE    mix-mask muls, PSUM evictions, reciprocals (recip emission is
         deferred one q-chunk so its repack DMA has already landed).
  GpSimd the row-sum DMA-repack / DRAM-bounce broadcast chain and the
         final normalize multiplies — latency-tolerant, keeps the DVE
         and Sync queues unblocked.
  Sync   bulk loads (merged one-DMA-per-tensor via 3D access patterns,
         weights queued ahead of x), rseg broadcasts, projT stores.

V carries an extra ones-column so row 64 of the AV accumulator is the
softmax row sum. The row-sum reciprocal is computed across 128
partitions (DMA repack [1,512] -> [128,4]) to dodge the DVE's serial
iterative-divide cost, broadcast via a DRAM bounce, and applied in one
fused multiply that also casts to fp16. The qkv projections for head
pair hp+1 are emitted between attention q-chunks as TensorE filler; the
output projection is emitted per q-chunk as soon as the last head pair
finishes that chunk, so it overlaps hp3's attention.
"""

import numpy as np

import concourse.bass as bass
import concourse.tile as tile
from concourse import bacc, mybir
from concourse import bass_utils

F32 = mybir.dt.float32
F16 = mybir.dt.float16
AF = mybir.ActivationFunctionType

B, T, D, H, HD = 4, 2048, 1024, 16, 64
N_CORES = 8
HL = 8          # heads per core (local)
CB = 8          # c (contraction) blocks of 128
TB = 16         # t blocks of 128
TC = 4          # t chunks of 512

_CACHED_NC = None


def _emit(tc, xT, wqkT, wvT, wpT, mixm, projT):
    nc = tc.nc
    from contextlib import ExitStack

    with ExitStack() as ctx:
        consts = ctx.enter_context(tc.tile_pool(name="consts", bufs=1))
        psum = ctx.enter_context(tc.tile_pool(name="psum", bufs=1, space="PSUM"))
        vtp = ctx.enter_context(tc.tile_pool(name="vtp", bufs=1))
        qkp = ctx.enter_context(tc.tile_pool(name="qkp", bufs=1))
        xsp = ctx.enter_context(tc.tile_pool(name="xsp", bufs=1))
        wvp = ctx.enter_context(tc.tile_pool(name="wvp", bufs=1))
        wqsp = ctx.enter_context(tc.tile_pool(name="wqsp", bufs=2))
        outup = ctx.enter_context(tc.tile_pool(name="outup", bufs=1))
        expp = ctx.enter_context(tc.tile_pool(name="expp", bufs=8))
        tmpp = ctx.enter_context(tc.tile_pool(name="tmpp", bufs=4))
        rpkp = ctx.enter_context(tc.tile_pool(name="rpkp", bufs=4))
        rsegp = ctx.enter_context(tc.tile_pool(name="rsegp", bufs=4))
        drp = ctx.enter_context(tc.tile_pool(name="drp", bufs=4, space="DRAM"))
        poutp = ctx.enter_context(tc.tile_pool(name="poutp", bufs=4))
        wpp = ctx.enter_context(tc.tile_pool(name="wpp", bufs=1))

        # ---- consts + merged bulk loads (weights ahead of x) ----
        mix_t = consts.tile([128, 128], F16, name="mix_t")
        nc.sync.dma_start(out=mix_t, in_=mixm)

        wq_tiles = {}

        def emit_qk_w(hp):
            """DMA the Q and K weight tiles for head pair hp (one DMA each)."""
            ws = []
            for i, fb in enumerate((hp, 4 + hp)):
                w_t = wqsp.tile([128, CB, 128], F16, tag=f"wq{i}",
                                name=f"w{fb}")
                nc.sync.dma_start(
                    out=w_t,
                    in_=bass.AP(tensor=wqkT.tensor,
                                offset=fb * CB * 128 * 128,
                                ap=[[128, 128], [128 * 128, CB], [1, 128]]))
                ws.append(w_t)
            wq_tiles[hp] = ws

        emit_qk_w(0)

        wvall = wvp.tile([128, CB, 512], F16, name="wvall")
        nc.sync.dma_start(
            out=wvall,
            in_=bass.AP(tensor=wvT.tensor, offset=0,
                        ap=[[512, 128], [128 * 512, CB], [1, 512]]))
        wv = [wvall[:, cb, :] for cb in range(CB)]

        xall = xsp.tile([128, CB, T], F16, name="xall")
        for tcc in range(TC):
            nc.sync.dma_start(
                out=xall[:, :, tcc * 512:(tcc + 1) * 512],
                in_=bass.AP(tensor=xT.tensor, offset=tcc * CB * 128 * 512,
                            ap=[[512, 128], [128 * 512, CB], [1, 512]]))
        xs = [xall[:, cb, :] for cb in range(CB)]

        wpall = wpp.tile([128, 4, 1024], F16, name="wpall")
        nc.sync.dma_start(
            out=wpall,
            in_=bass.AP(tensor=wpT.tensor, offset=0,
                        ap=[[1024, 128], [128 * 1024, 4], [1, 1024]]))
        wp = [wpall[:, j, :] for j in range(4)]

        # ---- V = x @ Wv^T, stored [128, 8 heads, 66] with ones col 64 ----
        vt = [None] * TB

        def emit_v(tbs):
            for tb in tbs:
                ps = psum.tile([128, 512], F32, tag="acc", bufs=2, name=f"vps{tb}")
                for cb in range(CB):
                    nc.tensor.matmul(
                        ps, lhsT=xs[cb][:, tb * 128:(tb + 1) * 128], rhs=wv[cb],
                        start=(cb == 0), stop=(cb == CB - 1))
                v_t = vtp.tile([128, HL, 66], F16, name=f"vt{tb}")
                nc.gpsimd.memset(v_t[:, :, 64:65], 1.0)
                nc.vector.tensor_copy(
                    out=v_t[:, :, 0:64],
                    in_=ps.rearrange("p (h d) -> p h d", h=HL))
                vt[tb] = v_t

        # Q^T / K^T tiles per head pair (rows 0:64 head A feats, 64:128 B)
        qq = [None] * 4
        kzs = [None] * 4
        outU = [outup.tile([128, T], F16, name=f"outU{j}") for j in range(4)]

        def emit_qk(hp, tccs):
            """Q^T and K^T projection for head pair hp, chunks tccs."""
            ws = wq_tiles[hp]
            if qq[hp] is None:
                qq[hp] = qkp.tile([128, T], F16, name=f"qq{hp}")
                kzs[hp] = qkp.tile([128, T], F16, name=f"kz{hp}")
            for tcc in tccs:
                for qk in range(2):
                    dst = (qq, kzs)[qk][hp]
                    ps = psum.tile([128, 512], F32, tag="acc", bufs=2,
                                   name=f"qkps{hp}_{qk}_{tcc}")
                    for cb in range(CB):
                        nc.tensor.matmul(
                            ps, lhsT=ws[qk][:, cb, :],
                            rhs=xs[cb][:, tcc * 512:(tcc + 1) * 512],
                            start=(cb == 0), stop=(cb == CB - 1))
                    nc.vector.tensor_copy(
                        out=dst[:, tcc * 512:(tcc + 1) * 512], in_=ps)

        def emit_attention(hp, qc, last=False):
            """Attention for head pair hp, q-chunk qc. Emits the QK/exp/AV
            loop plus immediate evictions; returns a closure that finishes
            the normalize chain (deferred so its DMAs are in flight)."""
            qT = qq[hp]
            kz = kzs[hp]
            nk = 4 * qc + 4
            otA = psum.tile([128, 512], F32, tag="otA", name=f"otA{hp}_{qc}")
            otB = psum.tile([128, 512], F32, tag="otB", name=f"otB{hp}_{qc}")
            for kb in range(nk):
                rb = kb - 4 * qc
                off = 128 * rb if rb > 0 else 0
                st = psum.tile([128, 2, 512], F32, tag="stag", bufs=2,
                               name=f"st{hp}_{qc}_{kb}")
                for sub in range(2):
                    r0 = 64 * sub
                    nc.tensor.matmul(
                        st[:, sub, off:512],
                        lhsT=kz[r0:r0 + 64, kb * 128:(kb + 1) * 128],
                        rhs=qT[r0:r0 + 64, qc * 512 + off:(qc + 1) * 512],
                        start=True, stop=True)
                ex = expp.tile([128, 2, 512], F16, tag="expst",
                               name=f"ex{hp}_{qc}_{kb}")
                nc.scalar.activation(out=ex[:, :, off:512],
                                     in_=st[:, :, off:512], func=AF.Exp)
                if rb >= 0:
                    # zero the mixed causal block (cols off..off+128), both heads
                    mixs = ex[:, :, off:off + 128]
                    mixb = bass.AP(
                        tensor=mix_t.tensor, offset=mix_t.offset,
                        ap=[list(mix_t.ap[0]), [0, 2], list(mix_t.ap[1])])
                    nc.vector.tensor_mul(mixs, mixs, mixb)
                for sub, ot in ((0, otA), (1, otB)):
                    nc.tensor.matmul(
                        ot[0:65, off:512],
                        lhsT=vt[kb][:, 2 * hp + sub, 0:65],
                        rhs=ex[:, sub, off:512],
                        start=(kb == 0), stop=(kb == nk - 1),
                        skip_group_check=True)
            # immediate evict: free the ot banks, start the repack DMAs
            tmps, rpks = [], []
            for sub, ot in ((0, otA), (1, otB)):
                tmp = tmpp.tile([65, 512], F32, tag="tmp",
                                name=f"tm{hp}_{qc}_{sub}")
                nc.vector.tensor_copy(out=tmp, in_=ot[0:65, :])
                rpk = rpkp.tile([128, 4], F32, tag="rpk",
                                name=f"rp{hp}_{qc}_{sub}")
                nc.gpsimd.dma_start(out=rpk, in_=tmp[64:65, :])
                tmps.append(tmp)
                rpks.append(rpk)

            def finish():
                # deferred: recip (repack long since landed), DRAM-bounce
                # broadcast, and the normalize multiply on GpSimd
                for sub in range(2):
                    tmp, rpk = tmps[sub], rpks[sub]
                    r0 = sub * 64
                    nc.vector.reciprocal(out=rpk, in_=rpk)
                    dr = drp.tile([1, 512], F32, tag="dr",
                                  name=f"dr{hp}_{qc}_{sub}")
                    nc.gpsimd.dma_start(out=dr, in_=rpk)
                    bc = bass.AP(tensor=dr.tensor, offset=dr.offset,
                                 ap=[[0, 64]] + [list(dd) for dd in dr.ap])
                    rseg = rsegp.tile([64, 512], F32, tag="rseg",
                                      name=f"rg{hp}_{qc}_{sub}")
                    nc.sync.dma_start(out=rseg, in_=bc)
                    eng = nc.vector if (last and sub == 0) else nc.gpsimd
                    eng.tensor_mul(
                        outU[hp][r0:r0 + 64, qc * 512:(qc + 1) * 512],
                        tmp[0:64, :], rseg)

            return finish

        # ---- partial projection: projT[o, t] = wpT.T @ outU, per t-chunk ----
        def emit_proj(tcc):
            for ob in range(8):
                ps = psum.tile([128, 512], F32, tag="acc", bufs=2,
                               name=f"pps{ob}_{tcc}")
                for j in range(4):
                    nc.tensor.matmul(
                        ps, lhsT=wp[j][:, ob * 128:(ob + 1) * 128],
                        rhs=outU[j][:, tcc * 512:(tcc + 1) * 512],
                        start=(j == 0), stop=(j == 3))
                po = poutp.tile([128, 512], F16, tag="pout", bufs=4,
                                name=f"po{ob}_{tcc}")
                if tcc == 3 and ob % 2 == 0:
                    nc.scalar.copy(out=po, in_=ps)
                else:
                    nc.vector.tensor_copy(out=po, in_=ps)
                nc.sync.dma_start(
                    out=projT[ob * 128:(ob + 1) * 128,
                              tcc * 512:(tcc + 1) * 512], in_=po)

        # ---- schedule ----
        pend = None

        def attn(hp, qc, last=False):
            nonlocal pend
            fin = emit_attention(hp, qc, last=last)
            if pend is not None:
                pend()
            pend = fin

        emit_v([0, 1, 2, 3])
        emit_qk(0, [0])
        attn(0, 0)
        emit_v([4, 5, 6, 7])
        emit_qk(0, [1])
        attn(0, 1)
        emit_v([8, 9, 10, 11])
        emit_qk(0, [2])
        attn(0, 2)
        emit_v([12, 13, 14, 15])
        emit_qk(0, [3])
        emit_qk_w(1)
        attn(0, 3)
        emit_qk(1, [0, 1])
        attn(1, 0)
        emit_qk(1, [2, 3])
        attn(1, 1)
        emit_qk_w(2)
        attn(1, 2)
        emit_qk(2, [0, 1])
        attn(1, 3)
        emit_qk(2, [2, 3])
        attn(2, 0)
        emit_qk_w(3)
        attn(2, 1)
        emit_qk(3, [0, 1])
        attn(2, 2)
        emit_qk(3, [2, 3])
        attn(2, 3)
        # hp3: output projection per q-chunk once that chunk's deferred
        # normalize (finish) has been emitted — attn(3, qc+1) flushes
        # finish(3, qc), so proj(qc) trails by one call
        attn(3, 0)
        attn(3, 1)
        emit_proj(0)
        attn(3, 2)
        emit_proj(1)
        pend()          # finish(3, 2) now; proj(2) can overlap attn(3, 3)
        pend = None
        emit_proj(2)
        attn(3, 3, last=True)
        pend()
        pend = None
        emit_proj(3)


def build_nc():
    global _CACHED_NC
    if _CACHED_NC is not None:
        return _CACHED_NC
    nc = bacc.Bacc("TRN2", target_bir_lowering=False, debug=False,
                   num_devices=N_CORES)
    xT = nc.dram_tensor("xT", [TC, CB, 128, 512], F16, kind="ExternalInput").ap()
    wqkT = nc.dram_tensor("wqkT", [8, CB, 128, 128], F16, kind="ExternalInput").ap()
    wvT = nc.dram_tensor("wvT", [D, 512], F16, kind="ExternalInput").ap()
    wpT = nc.dram_tensor("wpT", [512, D], F16, kind="ExternalInput").ap()
    mixm = nc.dram_tensor("mixm", [128, 128], F16, kind="ExternalInput").ap()
    projT = nc.dram_tensor("projT", [D, T], F16, kind="ExternalOutput").ap()

    with tile.TileContext(nc) as t:
        _emit(t, xT, wqkT, wvT, wpT, mixm, projT)
    nc.compile()
    _CACHED_NC = nc
    return nc


def make_in_maps(x, W_qkv, W_proj):
    x = np.asarray(x, dtype=np.float32)
    W_qkv = np.asarray(W_qkv, dtype=np.float32)
    W_proj = np.asarray(W_proj, dtype=np.float32)

    # mixed-block causal mask: keep (1.0) iff q >= k
    mixm = (np.arange(128)[None, :] >=
            np.arange(128)[:, None]).astype(np.float16)

    in_maps = []
    for core in range(N_CORES):
        b, half = core // 2, core % 2
        s = 512 * half
        # fold the 1/sqrt(HD) attention scale into the Q weights
        wq = W_qkv[s:s + 512] * np.float32(1.0 / np.sqrt(HD))
        wk = W_qkv[1024 + s:1024 + s + 512]
        wvv = W_qkv[2048 + s:2048 + s + 512]
        wcatT = np.ascontiguousarray(np.concatenate([wq, wk], axis=0).T)  # [c, f]
        wqkT = np.ascontiguousarray(
            wcatT.reshape(8, 128, 8, 128).transpose(2, 0, 1, 3))  # [fb, cb, c, f]
        xTb = np.ascontiguousarray(
            x[b].T.reshape(CB, 128, TC, 512).transpose(2, 0, 1, 3)
        ).astype(np.float16)  # [tcc, cb, 128, 512]
        in_maps.append({
            "xT": xTb,
            "wqkT": wqkT.astype(np.float16),
            "wvT": np.ascontiguousarray(wvv.T).astype(np.float16),
            "wpT": np.ascontiguousarray(W_proj[:, s:s + 512].T).astype(np.float16),
            "mixm": mixm,
        })
    return in_maps


def gather_output(results, b_proj):
    b_proj = np.asarray(b_proj, dtype=np.float32)
    out = np.empty((B, T, D), dtype=np.float32)
    for b in range(B):
        p = (results[2 * b]["projT"].astype(np.float32) +
             results[2 * b + 1]["projT"].astype(np.float32))  # [D, T]
        out[b] = p.T + b_proj[None, :]
    return out


def run(x, W_qkv, W_proj, b_proj, trace=False, tmpdir=None):
    nc = build_nc()
    in_maps = make_in_maps(x, W_qkv, W_proj)
    if trace:
        bass_utils.upload_artifacts = lambda d: d
    res = bass_utils.run_bass_kernel_spmd(
        nc, in_maps, core_ids=list(range(N_CORES)), trace=trace, tmpdir=tmpdir)
    return gather_output(res.results, b_proj), res


def kernel(x, W_qkv, W_proj, b_proj):
    out, _ = run(x, W_qkv, W_proj, b_proj)
    return out


# revision 15
# speedup vs baseline: 1.0432x; 1.0432x over previous
"""Multi-head causal self-attention on 8 Trainium2 NeuronCores.

Sharding: core = (batch b, head-half). Each of the 8 cores computes
attention for 8 of the 16 heads of one of the 4 batch elements, plus the
partial output projection over its 512 feature columns. Host sums the two
partial projections per batch and adds the bias.

All device tensors are kept transposed (feature-major) so every matmul
contraction lands on the partition axis:
  QK^T:  S^T[k,q] = K^T_blk.T @ Q^T_chunk           (contraction 64)
  AV:    outT[d,q] = V_ext_blk.T @ expS^T_blk       (contraction k=128)

QK^T runs both heads of a head-pair as two CONCURRENT row-tiled K=64
matmuls (head A in PE rows 0:64, head B in rows 64:128, tile_position
auto-derived from the operands' base partitions) writing the two halves
of one [128, 2, 512] PSUM stag tile — 2x the padded-K=128 scheme.
Diagonal-band tiles only compute/exp the live columns [off:512].

Engine-queue discipline (each queue is in-order; an op that waits on a
semaphore blocks everything behind it):
  ACT    exp only (the attention critical path).
  DVE    mix-mask muls, PSUM evictions, reciprocals (recip emission is
         deferred one q-chunk so its repack DMA has already landed).
  GpSimd the row-sum DMA-repack / DRAM-bounce broadcast chain and the
         final normalize multiplies — latency-tolerant, keeps the DVE
         and Sync queues unblocked.
  Sync   bulk loads (merged one-DMA-per-tensor via 3D access patterns,
         weights queued ahead of x), rseg broadcasts, projT stores.

V carries an extra ones-column so row 64 of the AV accumulator is the
softmax row sum. The row-sum reciprocal is computed across 128
partitions (DMA repack [1,512] -> [128,4]) to dodge the DVE's serial
iterative-divide cost, broadcast via a DRAM bounce, and applied in one
fused multiply that also casts to fp16. The qkv projections for head
pair hp+1 are emitted between attention q-chunks as TensorE filler; the
output projection is emitted per q-chunk as soon as the last head pair
finishes that chunk, so it overlaps hp3's attention.
"""

import numpy as np

import concourse.bass as bass
import concourse.tile as tile
from concourse import bacc, mybir
from concourse import bass_utils

F32 = mybir.dt.float32
F16 = mybir.dt.float16
AF = mybir.ActivationFunctionType

B, T, D, H, HD = 4, 2048, 1024, 16, 64
N_CORES = 8
HL = 8          # heads per core (local)
CB = 8          # c (contraction) blocks of 128
TB = 16         # t blocks of 128
TC = 4          # t chunks of 512

_CACHED_NC = None


def _emit(tc, xT, wqkT, wvT, wpT, mixm, projT):
    nc = tc.nc
    from contextlib import ExitStack

    with ExitStack() as ctx:
        consts = ctx.enter_context(tc.tile_pool(name="consts", bufs=1))
        psum = ctx.enter_context(tc.tile_pool(name="psum", bufs=1, space="PSUM"))
        vtp = ctx.enter_context(tc.tile_pool(name="vtp", bufs=1))
        qkp = ctx.enter_context(tc.tile_pool(name="qkp", bufs=1))
        xsp = ctx.enter_context(tc.tile_pool(name="xsp", bufs=1))
        wvp = ctx.enter_context(tc.tile_pool(name="wvp", bufs=1))
        wqsp = ctx.enter_context(tc.tile_pool(name="wqsp", bufs=2))
        outup = ctx.enter_context(tc.tile_pool(name="outup", bufs=1))
        expp = ctx.enter_context(tc.tile_pool(name="expp", bufs=8))
        tmpp = ctx.enter_context(tc.tile_pool(name="tmpp", bufs=4))
        rpkp = ctx.enter_context(tc.tile_pool(name="rpkp", bufs=4))
        rsegp = ctx.enter_context(tc.tile_pool(name="rsegp", bufs=4))
        drp = ctx.enter_context(tc.tile_pool(name="drp", bufs=4, space="DRAM"))
        poutp = ctx.enter_context(tc.tile_pool(name="poutp", bufs=4))
        wpp = ctx.enter_context(tc.tile_pool(name="wpp", bufs=1))

        # ---- consts + merged bulk loads (first-needed chunks first) ----
        xall = xsp.tile([128, CB, T], F16, name="xall")
        nc.sync.dma_start(
            out=xall[:, :, 0:512],
            in_=bass.AP(tensor=xT.tensor, offset=0,
                        ap=[[T, 128], [128 * T, CB], [1, 512]]))
        mix_t = consts.tile([128, 128], F16, name="mix_t")
        nc.sync.dma_start(out=mix_t, in_=mixm)

        wq_tiles = {}

        def emit_qk_w(hp):
            """DMA the Q and K weight tiles for head pair hp (one DMA each)."""
            ws = []
            for i, fb in enumerate((hp, 4 + hp)):
                w_t = wqsp.tile([128, CB, 128], F16, tag=f"wq{i}",
                                name=f"w{fb}")
                nc.sync.dma_start(
                    out=w_t,
                    in_=bass.AP(tensor=wqkT.tensor,
                                offset=fb * CB * 128 * 128,
                                ap=[[128, 128], [128 * 128, CB], [1, 128]]))
                ws.append(w_t)
            wq_tiles[hp] = ws

        emit_qk_w(0)

        wvall = wvp.tile([128, CB, 512], F16, name="wvall")
        nc.sync.dma_start(
            out=wvall,
            in_=bass.AP(tensor=wvT.tensor, offset=0,
                        ap=[[512, 128], [128 * 512, CB], [1, 512]]))
        wv = [wvall[:, cb, :] for cb in range(CB)]

        xall = xsp.tile([128, CB, T], F16, name="xall")
        for tcc in range(TC):
            nc.sync.dma_start(
                out=xall[:, :, tcc * 512:(tcc + 1) * 512],
                in_=bass.AP(tensor=xT.tensor, offset=tcc * CB * 128 * 512,
                            ap=[[512, 128], [128 * 512, CB], [1, 512]]))
        xs = [xall[:, cb, :] for cb in range(CB)]

        wpall = wpp.tile([128, 4, 1024], F16, name="wpall")
        nc.sync.dma_start(
            out=wpall,
            in_=bass.AP(tensor=wpT.tensor, offset=0,
                        ap=[[1024, 128], [128 * 1024, 4], [1, 1024]]))
        wp = [wpall[:, j, :] for j in range(4)]

        # ---- V = x @ Wv^T, stored [128, 8 heads, 66] with ones col 64 ----
        vt = [None] * TB

        def emit_v(tbs):
            for tb in tbs:
                ps = psum.tile([128, 512], F32, tag="acc", bufs=2, name=f"vps{tb}")
                for cb in range(CB):
                    nc.tensor.matmul(
                        ps, lhsT=xs[cb][:, tb * 128:(tb + 1) * 128], rhs=wv[cb],
                        start=(cb == 0), stop=(cb == CB - 1))
                v_t = vtp.tile([128, HL, 66], F16, name=f"vt{tb}")
                nc.gpsimd.memset(v_t[:, :, 64:65], 1.0)
                nc.vector.tensor_copy(
                    out=v_t[:, :, 0:64],
                    in_=ps.rearrange("p (h d) -> p h d", h=HL))
                vt[tb] = v_t

        # Q^T / K^T tiles per head pair (rows 0:64 head A feats, 64:128 B)
        qq = [None] * 4
        kzs = [None] * 4
        outU = [outup.tile([128, T], F16, name=f"outU{j}") for j in range(4)]

        def emit_qk(hp, tccs):
            """Q^T and K^T projection for head pair hp, chunks tccs."""
            ws = wq_tiles[hp]
            if qq[hp] is None:
                qq[hp] = qkp.tile([128, T], F16, name=f"qq{hp}")
                kzs[hp] = qkp.tile([128, T], F16, name=f"kz{hp}")
            for tcc in tccs:
                for qk in range(2):
                    dst = (qq, kzs)[qk][hp]
                    ps = psum.tile([128, 512], F32, tag="acc", bufs=2,
                                   name=f"qkps{hp}_{qk}_{tcc}")
                    for cb in range(CB):
                        nc.tensor.matmul(
                            ps, lhsT=ws[qk][:, cb, :],
                            rhs=xs[cb][:, tcc * 512:(tcc + 1) * 512],
                            start=(cb == 0), stop=(cb == CB - 1))
                    nc.vector.tensor_copy(
                        out=dst[:, tcc * 512:(tcc + 1) * 512], in_=ps)

        def emit_attention(hp, qc, last=False):
            """Attention for head pair hp, q-chunk qc. Emits the QK/exp/AV
            loop plus immediate evictions; returns a closure that finishes
            the normalize chain (deferred so its DMAs are in flight)."""
            qT = qq[hp]
            kz = kzs[hp]
            nk = 4 * qc + 4
            otA = psum.tile([128, 512], F32, tag="otA", name=f"otA{hp}_{qc}")
            otB = psum.tile([128, 512], F32, tag="otB", name=f"otB{hp}_{qc}")
            for kb in range(nk):
                rb = kb - 4 * qc
                off = 128 * rb if rb > 0 else 0
                st = psum.tile([128, 2, 512], F32, tag="stag", bufs=2,
                               name=f"st{hp}_{qc}_{kb}")
                for sub in range(2):
                    r0 = 64 * sub
                    nc.tensor.matmul(
                        st[:, sub, off:512],
                        lhsT=kz[r0:r0 + 64, kb * 128:(kb + 1) * 128],
                        rhs=qT[r0:r0 + 64, qc * 512 + off:(qc + 1) * 512],
                        start=True, stop=True)
                ex = expp.tile([128, 2, 512], F16, tag="expst",
                               name=f"ex{hp}_{qc}_{kb}")
                nc.scalar.activation(out=ex[:, :, off:512],
                                     in_=st[:, :, off:512], func=AF.Exp)
                if rb >= 0:
                    # zero the mixed causal block (cols off..off+128), both heads
                    mixs = ex[:, :, off:off + 128]
                    mixb = bass.AP(
                        tensor=mix_t.tensor, offset=mix_t.offset,
                        ap=[list(mix_t.ap[0]), [0, 2], list(mix_t.ap[1])])
                    nc.vector.tensor_mul(mixs, mixs, mixb)
                for sub, ot in ((0, otA), (1, otB)):
                    nc.tensor.matmul(
                        ot[0:65, off:512],
                        lhsT=vt[kb][:, 2 * hp + sub, 0:65],
                        rhs=ex[:, sub, off:512],
                        start=(kb == 0), stop=(kb == nk - 1),
                        skip_group_check=True)
            # immediate evict: free the ot banks, start the repack DMAs
            tmps, rpks = [], []
            for sub, ot in ((0, otA), (1, otB)):
                tmp = tmpp.tile([65, 512], F32, tag="tmp",
                                name=f"tm{hp}_{qc}_{sub}")
                nc.vector.tensor_copy(out=tmp, in_=ot[0:65, :])
                rpk = rpkp.tile([128, 4], F32, tag="rpk",
                                name=f"rp{hp}_{qc}_{sub}")
                nc.gpsimd.dma_start(out=rpk, in_=tmp[64:65, :])
                tmps.append(tmp)
                rpks.append(rpk)

            def finish():
                # deferred: recip (repack long since landed), DRAM-bounce
                # broadcast, and the normalize multiply on GpSimd
                for sub in range(2):
                    tmp, rpk = tmps[sub], rpks[sub]
                    r0 = sub * 64
                    nc.vector.reciprocal(out=rpk, in_=rpk)
                    dr = drp.tile([1, 512], F32, tag="dr",
                                  name=f"dr{hp}_{qc}_{sub}")
                    nc.gpsimd.dma_start(out=dr, in_=rpk)
                    bc = bass.AP(tensor=dr.tensor, offset=dr.offset,
                                 ap=[[0, 64]] + [list(dd) for dd in dr.ap])
                    rseg = rsegp.tile([64, 512], F32, tag="rseg",
                                      name=f"rg{hp}_{qc}_{sub}")
                    nc.sync.dma_start(out=rseg, in_=bc)
                    eng = nc.vector if (last and sub == 0) else nc.gpsimd
                    eng.tensor_mul(
                        outU[hp][r0:r0 + 64, qc * 512:(qc + 1) * 512],
                        tmp[0:64, :], rseg)

            return finish

        # ---- partial projection: projT[o, t] = wpT.T @ outU, per t-chunk ----
        def emit_proj(tcc):
            for ob in range(8):
                ps = psum.tile([128, 512], F32, tag="acc", bufs=2,
                               name=f"pps{ob}_{tcc}")
                for j in range(4):
                    nc.tensor.matmul(
                        ps, lhsT=wp[j][:, ob * 128:(ob + 1) * 128],
                        rhs=outU[j][:, tcc * 512:(tcc + 1) * 512],
                        start=(j == 0), stop=(j == 3))
                po = poutp.tile([128, 512], F16, tag="pout", bufs=4,
                                name=f"po{ob}_{tcc}")
                if tcc == 3 and ob % 2 == 0:
                    nc.scalar.copy(out=po, in_=ps)
                else:
                    nc.vector.tensor_copy(out=po, in_=ps)
                nc.sync.dma_start(
                    out=projT[ob * 128:(ob + 1) * 128,
                              tcc * 512:(tcc + 1) * 512], in_=po)

        # ---- schedule ----
        pend = None

        def attn(hp, qc, last=False):
            nonlocal pend
            fin = emit_attention(hp, qc, last=last)
            if pend is not None:
                pend()
            pend = fin

        emit_v([0, 1, 2, 3])
        emit_qk(0, [0])
        attn(0, 0)
        emit_v([4, 5, 6, 7])
        emit_qk(0, [1])
        attn(0, 1)
        emit_v([8, 9, 10, 11])
        emit_qk(0, [2])
        attn(0, 2)
        emit_v([12, 13, 14, 15])
        emit_qk(0, [3])
        emit_qk_w(1)
        attn(0, 3)
        emit_qk(1, [0, 1])
        attn(1, 0)
        emit_qk(1, [2, 3])
        attn(1, 1)
        emit_qk_w(2)
        attn(1, 2)
        emit_qk(2, [0, 1])
        attn(1, 3)
        emit_qk(2, [2, 3])
        attn(2, 0)
        emit_qk_w(3)
        attn(2, 1)
        emit_qk(3, [0, 1])
        attn(2, 2)
        emit_qk(3, [2, 3])
        attn(2, 3)
        # hp3: output projection per q-chunk once that chunk's deferred
        # normalize (finish) has been emitted — attn(3, qc+1) flushes
        # finish(3, qc), so proj(qc) trails by one call
        attn(3, 0)
        attn(3, 1)
        emit_proj(0)
        attn(3, 2)
        emit_proj(1)
        attn(3, 3, last=True)
        pend()
        pend = None
        emit_proj(2)
        emit_proj(3)


def build_nc():
    global _CACHED_NC
    if _CACHED_NC is not None:
        return _CACHED_NC
    nc = bacc.Bacc("TRN2", target_bir_lowering=False, debug=False,
                   num_devices=N_CORES)
    xT = nc.dram_tensor("xT", [TC, CB, 128, 512], F16, kind="ExternalInput").ap()
    wqkT = nc.dram_tensor("wqkT", [8, CB, 128, 128], F16, kind="ExternalInput").ap()
    wvT = nc.dram_tensor("wvT", [D, 512], F16, kind="ExternalInput").ap()
    wpT = nc.dram_tensor("wpT", [512, D], F16, kind="ExternalInput").ap()
    mixm = nc.dram_tensor("mixm", [128, 128], F16, kind="ExternalInput").ap()
    projT = nc.dram_tensor("projT", [D, T], F16, kind="ExternalOutput").ap()

    with tile.TileContext(nc) as t:
        _emit(t, xT, wqkT, wvT, wpT, mixm, projT)
    nc.compile()
    _CACHED_NC = nc
    return nc


def make_in_maps(x, W_qkv, W_proj):
    x = np.asarray(x, dtype=np.float32)
    W_qkv = np.asarray(W_qkv, dtype=np.float32)
    W_proj = np.asarray(W_proj, dtype=np.float32)

    # mixed-block causal mask: keep (1.0) iff q >= k
    mixm = (np.arange(128)[None, :] >=
            np.arange(128)[:, None]).astype(np.float16)

    in_maps = []
    for core in range(N_CORES):
        b, half = core // 2, core % 2
        s = 512 * half
        # fold the 1/sqrt(HD) attention scale into the Q weights
        wq = W_qkv[s:s + 512] * np.float32(1.0 / np.sqrt(HD))
        wk = W_qkv[1024 + s:1024 + s + 512]
        wvv = W_qkv[2048 + s:2048 + s + 512]
        wcatT = np.ascontiguousarray(np.concatenate([wq, wk], axis=0).T)  # [c, f]
        wqkT = np.ascontiguousarray(
            wcatT.reshape(8, 128, 8, 128).transpose(2, 0, 1, 3))  # [fb, cb, c, f]
        xTb = np.ascontiguousarray(
            x[b].T.reshape(CB, 128, TC, 512).transpose(2, 0, 1, 3)
        ).astype(np.float16)  # [tcc, cb, 128, 512]
        in_maps.append({
            "xT": xTb,
            "wqkT": wqkT.astype(np.float16),
            "wvT": np.ascontiguousarray(wvv.T).astype(np.float16),
            "wpT": np.ascontiguousarray(W_proj[:, s:s + 512].T).astype(np.float16),
            "mixm": mixm,
        })
    return in_maps


def gather_output(results, b_proj):
    b_proj = np.asarray(b_proj, dtype=np.float32)
    out = np.empty((B, T, D), dtype=np.float32)
    for b in range(B):
        p = (results[2 * b]["projT"].astype(np.float32) +
             results[2 * b + 1]["projT"].astype(np.float32))  # [D, T]
        out[b] = p.T + b_proj[None, :]
    return out


def run(x, W_qkv, W_proj, b_proj, trace=False, tmpdir=None):
    nc = build_nc()
    in_maps = make_in_maps(x, W_qkv, W_proj)
    if trace:
        bass_utils.upload_artifacts = lambda d: d
    res = bass_utils.run_bass_kernel_spmd(
        nc, in_maps, core_ids=list(range(N_CORES)), trace=trace, tmpdir=tmpdir)
    return gather_output(res.results, b_proj), res


def kernel(x, W_qkv, W_proj, b_proj):
    out, _ = run(x, W_qkv, W_proj, b_proj)
    return out


# revision 16
# speedup vs baseline: 1.0567x; 1.0129x over previous
"""Multi-head causal self-attention on 8 Trainium2 NeuronCores.

Sharding: core = (batch b, head-half). Each of the 8 cores computes
attention for 8 of the 16 heads of one of the 4 batch elements, plus the
partial output projection over its 512 feature columns. Host sums the two
partial projections per batch and adds the bias.

All device tensors are kept transposed (feature-major) so every matmul
contraction lands on the partition axis:
  QK^T:  S^T[k,q] = K^T_blk.T @ Q^T_chunk           (contraction 64)
  AV:    outT[d,q] = V_ext_blk.T @ expS^T_blk       (contraction k=128)

QK^T runs both heads of a head-pair as two CONCURRENT row-tiled K=64
matmuls (head A in PE rows 0:64, head B in rows 64:128, tile_position
auto-derived from the operands' base partitions) writing the two halves
of one [128, 2, 512] PSUM stag tile — 2x the padded-K=128 scheme.
Diagonal-band tiles only compute/exp the live columns [off:512].

Engine-queue discipline (each queue is in-order; an op that waits on a
semaphore blocks everything behind it):
  ACT    exp only (the attention critical path).
  DVE    mix-mask muls, PSUM evictions, reciprocals (recip emission is
         deferred one q-chunk so its repack DMA has already landed).
  GpSimd the row-sum DMA-repack / DRAM-bounce broadcast chain and the
         final normalize multiplies — latency-tolerant, keeps the DVE
         and Sync queues unblocked.
  Sync   bulk loads (merged one-DMA-per-tensor via 3D access patterns,
         weights queued ahead of x), rseg broadcasts, projT stores.

V carries an extra ones-column so row 64 of the AV accumulator is the
softmax row sum. The row-sum reciprocal is computed across 128
partitions (DMA repack [1,512] -> [128,4]) to dodge the DVE's serial
iterative-divide cost, broadcast via a DRAM bounce, and applied in one
fused multiply that also casts to fp16. The qkv projections for head
pair hp+1 are emitted between attention q-chunks as TensorE filler; the
output projection is emitted per q-chunk as soon as the last head pair
finishes that chunk, so it overlaps hp3's attention.
"""

import numpy as np

import concourse.bass as bass
import concourse.tile as tile
from concourse import bacc, mybir
from concourse import bass_utils

F32 = mybir.dt.float32
F16 = mybir.dt.float16
AF = mybir.ActivationFunctionType

B, T, D, H, HD = 4, 2048, 1024, 16, 64
N_CORES = 8
HL = 8          # heads per core (local)
CB = 8          # c (contraction) blocks of 128
TB = 16         # t blocks of 128
TC = 4          # t chunks of 512

_CACHED_NC = None


def _emit(tc, xT, wqkT, wvT, wpT, mixm, projT):
    nc = tc.nc
    from contextlib import ExitStack

    with ExitStack() as ctx:
        consts = ctx.enter_context(tc.tile_pool(name="consts", bufs=1))
        psum = ctx.enter_context(tc.tile_pool(name="psum", bufs=1, space="PSUM"))
        vtp = ctx.enter_context(tc.tile_pool(name="vtp", bufs=1))
        qkp = ctx.enter_context(tc.tile_pool(name="qkp", bufs=1))
        xsp = ctx.enter_context(tc.tile_pool(name="xsp", bufs=1))
        wvp = ctx.enter_context(tc.tile_pool(name="wvp", bufs=1))
        wqsp = ctx.enter_context(tc.tile_pool(name="wqsp", bufs=2))
        outup = ctx.enter_context(tc.tile_pool(name="outup", bufs=1))
        expp = ctx.enter_context(tc.tile_pool(name="expp", bufs=8))
        tmpp = ctx.enter_context(tc.tile_pool(name="tmpp", bufs=6))
        rpkp = ctx.enter_context(tc.tile_pool(name="rpkp", bufs=6))
        rsegp = ctx.enter_context(tc.tile_pool(name="rsegp", bufs=6))
        drp = ctx.enter_context(tc.tile_pool(name="drp", bufs=6, space="DRAM"))
        poutp = ctx.enter_context(tc.tile_pool(name="poutp", bufs=6))
        wpp = ctx.enter_context(tc.tile_pool(name="wpp", bufs=1))

        # ---- consts + merged bulk loads (first-needed chunks first) ----
        xall = xsp.tile([128, CB, T], F16, name="xall")
        nc.sync.dma_start(
            out=xall[:, :, 0:512],
            in_=bass.AP(tensor=xT.tensor, offset=0,
                        ap=[[T, 128], [128 * T, CB], [1, 512]]))
        mix_t = consts.tile([128, 128], F16, name="mix_t")
        nc.sync.dma_start(out=mix_t, in_=mixm)

        wq_tiles = {}

        def emit_qk_w(hp):
            """DMA the Q and K weight tiles for head pair hp (one DMA each)."""
            ws = []
            for i, fb in enumerate((hp, 4 + hp)):
                w_t = wqsp.tile([128, CB, 128], F16, tag=f"wq{i}",
                                name=f"w{fb}")
                nc.sync.dma_start(
                    out=w_t,
                    in_=bass.AP(tensor=wqkT.tensor,
                                offset=fb * CB * 128 * 128,
                                ap=[[128, 128], [128 * 128, CB], [1, 128]]))
                ws.append(w_t)
            wq_tiles[hp] = ws

        emit_qk_w(0)

        wvall = wvp.tile([128, CB, 512], F16, name="wvall")
        nc.sync.dma_start(
            out=wvall,
            in_=bass.AP(tensor=wvT.tensor, offset=0,
                        ap=[[512, 128], [128 * 512, CB], [1, 512]]))
        wv = [wvall[:, cb, :] for cb in range(CB)]

        xall = xsp.tile([128, CB, T], F16, name="xall")
        for tcc in range(TC):
            nc.sync.dma_start(
                out=xall[:, :, tcc * 512:(tcc + 1) * 512],
                in_=bass.AP(tensor=xT.tensor, offset=tcc * CB * 128 * 512,
                            ap=[[512, 128], [128 * 512, CB], [1, 512]]))
        xs = [xall[:, cb, :] for cb in range(CB)]

        wpall = wpp.tile([128, 4, 1024], F16, name="wpall")
        nc.sync.dma_start(
            out=wpall,
            in_=bass.AP(tensor=wpT.tensor, offset=0,
                        ap=[[1024, 128], [128 * 1024, 4], [1, 1024]]))
        wp = [wpall[:, j, :] for j in range(4)]

        # ---- V = x @ Wv^T, stored [128, 8 heads, 66] with ones col 64 ----
        vt = [None] * TB

        def emit_v(tbs):
            for tb in tbs:
                ps = psum.tile([128, 512], F32, tag="acc", bufs=2, name=f"vps{tb}")
                for cb in range(CB):
                    nc.tensor.matmul(
                        ps, lhsT=xs[cb][:, tb * 128:(tb + 1) * 128], rhs=wv[cb],
                        start=(cb == 0), stop=(cb == CB - 1))
                v_t = vtp.tile([128, HL, 66], F16, name=f"vt{tb}")
                nc.gpsimd.memset(v_t[:, :, 64:65], 1.0)
                nc.vector.tensor_copy(
                    out=v_t[:, :, 0:64],
                    in_=ps.rearrange("p (h d) -> p h d", h=HL))
                vt[tb] = v_t

        # Q^T / K^T tiles per head pair (rows 0:64 head A feats, 64:128 B)
        qq = [None] * 4
        kzs = [None] * 4
        outU = [outup.tile([128, T], F16, name=f"outU{j}") for j in range(4)]

        def emit_qk(hp, tccs):
            """Q^T and K^T projection for head pair hp, chunks tccs."""
            ws = wq_tiles[hp]
            if qq[hp] is None:
                qq[hp] = qkp.tile([128, T], F16, name=f"qq{hp}")
                kzs[hp] = qkp.tile([128, T], F16, name=f"kz{hp}")
            for tcc in tccs:
                for qk in range(2):
                    dst = (qq, kzs)[qk][hp]
                    ps = psum.tile([128, 512], F32, tag="acc", bufs=2,
                                   name=f"qkps{hp}_{qk}_{tcc}")
                    for cb in range(CB):
                        nc.tensor.matmul(
                            ps, lhsT=ws[qk][:, cb, :],
                            rhs=xs[cb][:, tcc * 512:(tcc + 1) * 512],
                            start=(cb == 0), stop=(cb == CB - 1))
                    nc.vector.tensor_copy(
                        out=dst[:, tcc * 512:(tcc + 1) * 512], in_=ps)

        def emit_attention(hp, qc, last=False):
            """Attention for head pair hp, q-chunk qc. Emits the QK/exp/AV
            loop plus immediate evictions; returns a closure that finishes
            the normalize chain (deferred so its DMAs are in flight)."""
            qT = qq[hp]
            kz = kzs[hp]
            nk = 4 * qc + 4
            otA = psum.tile([128, 512], F32, tag="otA", name=f"otA{hp}_{qc}")
            otB = psum.tile([128, 512], F32, tag="otB", name=f"otB{hp}_{qc}")
            for kb in range(nk):
                rb = kb - 4 * qc
                off = 128 * rb if rb > 0 else 0
                st = psum.tile([128, 2, 512], F32, tag="stag", bufs=2,
                               name=f"st{hp}_{qc}_{kb}")
                for sub in range(2):
                    r0 = 64 * sub
                    nc.tensor.matmul(
                        st[:, sub, off:512],
                        lhsT=kz[r0:r0 + 64, kb * 128:(kb + 1) * 128],
                        rhs=qT[r0:r0 + 64, qc * 512 + off:(qc + 1) * 512],
                        start=True, stop=True)
                ex = expp.tile([128, 2, 512], F16, tag="expst",
                               name=f"ex{hp}_{qc}_{kb}")
                nc.scalar.activation(out=ex[:, :, off:512],
                                     in_=st[:, :, off:512], func=AF.Exp)
                if rb >= 0:
                    # zero the mixed causal block (cols off..off+128), both heads
                    mixs = ex[:, :, off:off + 128]
                    mixb = bass.AP(
                        tensor=mix_t.tensor, offset=mix_t.offset,
                        ap=[list(mix_t.ap[0]), [0, 2], list(mix_t.ap[1])])
                    nc.vector.tensor_mul(mixs, mixs, mixb)
                for sub, ot in ((0, otA), (1, otB)):
                    nc.tensor.matmul(
                        ot[0:65, off:512],
                        lhsT=vt[kb][:, 2 * hp + sub, 0:65],
                        rhs=ex[:, sub, off:512],
                        start=(kb == 0), stop=(kb == nk - 1),
                        skip_group_check=True)
            # immediate evict: free the ot banks, start the repack DMAs
            tmps, rpks = [], []
            for sub, ot in ((0, otA), (1, otB)):
                tmp = tmpp.tile([65, 512], F32, tag="tmp",
                                name=f"tm{hp}_{qc}_{sub}")
                nc.vector.tensor_copy(out=tmp, in_=ot[0:65, :])
                rpk = rpkp.tile([128, 4], F32, tag="rpk",
                                name=f"rp{hp}_{qc}_{sub}")
                nc.gpsimd.dma_start(out=rpk, in_=tmp[64:65, :])
                tmps.append(tmp)
                rpks.append(rpk)

            def finish():
                # deferred: recip (repack long since landed), DRAM-bounce
                # broadcast, and the normalize multiply on GpSimd
                for sub in range(2):
                    tmp, rpk = tmps[sub], rpks[sub]
                    r0 = sub * 64
                    nc.vector.reciprocal(out=rpk, in_=rpk)
                    dr = drp.tile([1, 512], F32, tag="dr",
                                  name=f"dr{hp}_{qc}_{sub}")
                    nc.gpsimd.dma_start(out=dr, in_=rpk)
                    bc = bass.AP(tensor=dr.tensor, offset=dr.offset,
                                 ap=[[0, 64]] + [list(dd) for dd in dr.ap])
                    rseg = rsegp.tile([64, 512], F32, tag="rseg",
                                      name=f"rg{hp}_{qc}_{sub}")
                    nc.sync.dma_start(out=rseg, in_=bc)
                    eng = nc.vector if (last and sub == 0) else nc.gpsimd
                    eng.tensor_mul(
                        outU[hp][r0:r0 + 64, qc * 512:(qc + 1) * 512],
                        tmp[0:64, :], rseg)

            return finish

        # ---- partial projection: projT[o, t] = wpT.T @ outU, per t-chunk ----
        def emit_proj(tcc):
            for ob in range(8):
                ps = psum.tile([128, 512], F32, tag="acc", bufs=2,
                               name=f"pps{ob}_{tcc}")
                for j in range(4):
                    nc.tensor.matmul(
                        ps, lhsT=wp[j][:, ob * 128:(ob + 1) * 128],
                        rhs=outU[j][:, tcc * 512:(tcc + 1) * 512],
                        start=(j == 0), stop=(j == 3))
                po = poutp.tile([128, 512], F16, tag="pout", bufs=4,
                                name=f"po{ob}_{tcc}")
                if tcc == 3 and ob % 2 == 0:
                    nc.scalar.copy(out=po, in_=ps)
                else:
                    nc.vector.tensor_copy(out=po, in_=ps)
                nc.sync.dma_start(
                    out=projT[ob * 128:(ob + 1) * 128,
                              tcc * 512:(tcc + 1) * 512], in_=po)

        # ---- schedule ----
        pend = None

        def attn(hp, qc, last=False):
            nonlocal pend
            fin = emit_attention(hp, qc, last=last)
            if pend is not None:
                pend()
            pend = fin

        emit_v([0, 1, 2, 3])
        emit_qk(0, [0])
        attn(0, 0)
        emit_v([4, 5, 6, 7])
        emit_qk(0, [1])
        attn(0, 1)
        emit_v([8, 9, 10, 11])
        emit_qk(0, [2])
        attn(0, 2)
        emit_v([12, 13, 14, 15])
        emit_qk(0, [3])
        emit_qk_w(1)
        attn(0, 3)
        emit_qk(1, [0, 1])
        attn(1, 0)
        emit_qk(1, [2, 3])
        attn(1, 1)
        emit_qk_w(2)
        attn(1, 2)
        emit_qk(2, [0, 1])
        attn(1, 3)
        emit_qk(2, [2, 3])
        attn(2, 0)
        emit_qk_w(3)
        attn(2, 1)
        emit_qk(3, [0, 1])
        attn(2, 2)
        emit_qk(3, [2, 3])
        attn(2, 3)
        # hp3: output projection per q-chunk once that chunk's deferred
        # normalize (finish) has been emitted — attn(3, qc+1) flushes
        # finish(3, qc), so proj(qc) trails by one call
        attn(3, 0)
        attn(3, 1)
        emit_proj(0)
        attn(3, 2)
        emit_proj(1)
        attn(3, 3, last=True)
        pend()
        pend = None
        emit_proj(2)
        emit_proj(3)


def build_nc():
    global _CACHED_NC
    if _CACHED_NC is not None:
        return _CACHED_NC
    nc = bacc.Bacc("TRN2", target_bir_lowering=False, debug=False,
                   num_devices=N_CORES)
    xT = nc.dram_tensor("xT", [TC, CB, 128, 512], F16, kind="ExternalInput").ap()
    wqkT = nc.dram_tensor("wqkT", [8, CB, 128, 128], F16, kind="ExternalInput").ap()
    wvT = nc.dram_tensor("wvT", [D, 512], F16, kind="ExternalInput").ap()
    wpT = nc.dram_tensor("wpT", [512, D], F16, kind="ExternalInput").ap()
    mixm = nc.dram_tensor("mixm", [128, 128], F16, kind="ExternalInput").ap()
    projT = nc.dram_tensor("projT", [D, T], F16, kind="ExternalOutput").ap()

    with tile.TileContext(nc) as t:
        _emit(t, xT, wqkT, wvT, wpT, mixm, projT)
    nc.compile()
    _CACHED_NC = nc
    return nc


def make_in_maps(x, W_qkv, W_proj):
    x = np.asarray(x, dtype=np.float32)
    W_qkv = np.asarray(W_qkv, dtype=np.float32)
    W_proj = np.asarray(W_proj, dtype=np.float32)

    # mixed-block causal mask: keep (1.0) iff q >= k
    mixm = (np.arange(128)[None, :] >=
            np.arange(128)[:, None]).astype(np.float16)

    in_maps = []
    for core in range(N_CORES):
        b, half = core // 2, core % 2
        s = 512 * half
        # fold the 1/sqrt(HD) attention scale into the Q weights
        wq = W_qkv[s:s + 512] * np.float32(1.0 / np.sqrt(HD))
        wk = W_qkv[1024 + s:1024 + s + 512]
        wvv = W_qkv[2048 + s:2048 + s + 512]
        wcatT = np.ascontiguousarray(np.concatenate([wq, wk], axis=0).T)  # [c, f]
        wqkT = np.ascontiguousarray(
            wcatT.reshape(8, 128, 8, 128).transpose(2, 0, 1, 3))  # [fb, cb, c, f]
        xTb = np.ascontiguousarray(
            x[b].T.reshape(CB, 128, TC, 512).transpose(2, 0, 1, 3)
        ).astype(np.float16)  # [tcc, cb, 128, 512]
        in_maps.append({
            "xT": xTb,
            "wqkT": wqkT.astype(np.float16),
            "wvT": np.ascontiguousarray(wvv.T).astype(np.float16),
            "wpT": np.ascontiguousarray(W_proj[:, s:s + 512].T).astype(np.float16),
            "mixm": mixm,
        })
    return in_maps


def gather_output(results, b_proj):
    b_proj = np.asarray(b_proj, dtype=np.float32)
    out = np.empty((B, T, D), dtype=np.float32)
    for b in range(B):
        p = (results[2 * b]["projT"].astype(np.float32) +
             results[2 * b + 1]["projT"].astype(np.float32))  # [D, T]
        out[b] = p.T + b_proj[None, :]
    return out


def run(x, W_qkv, W_proj, b_proj, trace=False, tmpdir=None):
    nc = build_nc()
    in_maps = make_in_maps(x, W_qkv, W_proj)
    if trace:
        bass_utils.upload_artifacts = lambda d: d
    res = bass_utils.run_bass_kernel_spmd(
        nc, in_maps, core_ids=list(range(N_CORES)), trace=trace, tmpdir=tmpdir)
    return gather_output(res.results, b_proj), res


def kernel(x, W_qkv, W_proj, b_proj):
    out, _ = run(x, W_qkv, W_proj, b_proj)
    return out
